# revision 10
# baseline (speedup 1.0000x reference)
"""Trainium2 Bass kernel for nn_LutLinear (BCQ/LUT-quantized linear layer).

Math (K=4096, N=4096, WBIT=3, GROUP=128, APOT=3):
  bits[k, b, n]  = bit (k%32) of binaryWeight[k//32, b, n]
  B              = 2*bits - 1                        (in {-1, +1})
  scale[n, b, g] = sum_a 2^alpha[n, b, g, a]
  out[n] = sum_{g,b} scale[n,b,g] * (sum_{k in group g} x[k] * B[k,b,n]) + bias[n]

Strategy (tensor-parallel over N, 8 cores, N'=512 each):
  * binaryWeight arrives as six DRAM-contiguous (b, col-half) blocks on the
    3 DMA queues; the h0 blocks land ~2us before the full tensor would,
    letting the DVE unpack (the pacing chain) start that much earlier.
  * Bit-unpack on DVE: per (s, half): int32 (shift + AND 0x40404040), the
    masked word bitcast to fp8e4m3 holds 2.0*bit in each byte.
  * PE: 2*32*4 = 256 matmuls as 64 CONCURRENT TRIPLES (M=32 col-tiling over
    b) of 256 columns each, riding just behind the unpack chain.
  * scale: alpha as int8; GPSIMD computes (a*8+56) whose fp8e4m3 bit pattern
    is exactly 2^a; DVE sums the 3 APoT terms after its unpacks.
  * Tail (transpose-free): prod[0:96] = (psum96 - S_g) * scaleT fused on DVE
    (bf16 out); prod row 96 = bias row (DMA'd); reduce over 97 partitions
    via 4 ones-matmuls po[n'chunk] = prod_chunk^T @ ones; ACT copy; DMA out.
"""

import os
import sys

for _p in ("/opt/trn_rl_repo", "/opt/pypackages"):
    if os.path.isdir(_p) and _p not in sys.path:
        sys.path.insert(0, _p)

from contextlib import ExitStack

import ml_dtypes
import numpy as np

import concourse.bass as bass
import concourse.tile as tile
from concourse import bacc, mybir
from concourse._compat import with_exitstack
from concourse.bass_utils import run_bass_kernel_spmd

K = 4096
N = 4096
GROUP = 128
WBIT = 3
NUM_APOT = 3
G = K // GROUP          # 32 groups
NCORES = 8
NS = N // NCORES        # 512 output features per core
NH = NS // 2            # 256: column half
NBLK = NS // 128        # 4 partition-blocks of n'
WORDS = K // 32         # 128 packed words per (b, n)
Q = WBIT * G            # 96 (b, g) rows
WC = WBIT * NS          # 1536 packed-word columns
HC = WBIT * NH          # 768 cols per half tile
ACOLS = NS * NUM_APOT   # alpha cols per q-row: (n, a)

_CACHE = {}


@with_exitstack
def _build_kernel_body(ctx: ExitStack, tc):
    nc = tc.nc
    f32 = mybir.dt.float32
    i32 = mybir.dt.int32
    i8 = mybir.dt.int8
    bf16 = mybir.dt.bfloat16
    f8 = mybir.dt.float8e4

    # bwc rows (2b+h)*128 + w, cols n-in-half: each (b,h) block is contiguous
    bwc = nc.dram_tensor("bwc", [6 * WORDS, NH], i32, kind="ExternalInput")
    xall = nc.dram_tensor("xall", [WORDS, G * G], bf16, kind="ExternalInput")
    alc = nc.dram_tensor("alc", [Q, ACOLS], i8, kind="ExternalInput")
    cst = nc.dram_tensor("cst", [128, 5], f32, kind="ExternalInput")
    brow = nc.dram_tensor("brow", [1, NS], bf16, kind="ExternalInput")
    out = nc.dram_tensor("out", [128, NBLK], f32, kind="ExternalOutput")

    sb = ctx.enter_context(tc.tile_pool(name="sb", bufs=1))
    psum = ctx.enter_context(tc.tile_pool(name="psum", bufs=1, space="PSUM"))

    wsbH = [sb.tile([WORDS, HC], i32, tag=f"wsbH{h}", name=f"wsbH{h}")
            for h in range(2)]
    xall_sb = sb.tile([WORDS, G * G], bf16)
    alsb = sb.tile([Q, ACOLS], i8)
    csb = sb.tile([128, 5], f32)
    prod = sb.tile([Q + 1, NS], bf16)   # rows 0..95 DVE; row 96 = bias (DMA)

    # --- DMAs on the 3 queues: h0 bw blocks first (they gate the unpack),
    # xall slices timed to land just before their triples, h1 blocks next,
    # alpha last (needed only at the tail).
    def bwblk(b, h):
        return bwc[(2 * b + h) * WORDS:(2 * b + h + 1) * WORDS, :]

    nc.sync.dma_start(wsbH[0][:, 0:NH], bwblk(0, 0))
    nc.scalar.dma_start(wsbH[0][:, NH:2 * NH], bwblk(1, 0))
    nc.gpsimd.dma_start(wsbH[0][:, 2 * NH:3 * NH], bwblk(2, 0))

    nc.sync.dma_start(xall_sb[:, 0:384], xall[:, 0:384])          # s=0..2
    nc.scalar.dma_start(xall_sb[:, 384:1024], xall[:, 384:1024])  # s=3..7
    nc.gpsimd.dma_start(alsb[:], alc[:, :])

    nc.sync.dma_start(wsbH[1][:, 0:NH], bwblk(0, 1))
    nc.scalar.dma_start(wsbH[1][:, NH:2 * NH], bwblk(1, 1))
    nc.gpsimd.dma_start(wsbH[1][:, 2 * NH:3 * NH], bwblk(2, 1))

    nc.sync.dma_start(csb[:], cst[:, :])
    nc.sync.dma_start(prod[Q:Q + 1, :], brow[:, :])

    onesb = sb.tile([Q + 1, 1], bf16)
    nc.vector.memset(onesb[:], 1.0)
    warm = sb.tile([128, 544], bf16)
    nc.vector.memset(warm[:], 0.0)

    # --- PE pre-warm: ~2.6us of activity toward the HAM un-throttle ----------
    psw = psum.tile([32, NS], f32, tag="psw", name="psw")
    for _ in range(6):
        nc.tensor.matmul(
            psw[:, :], warm[:, :32], warm[:, 32:544], start=True, stop=True
        )

    # --- scale: GPSIMD (a*8+56) == fp8e4m3 bit pattern of 2^a ----------------
    ae = sb.tile([Q, ACOLS], i8)
    nc.gpsimd.tensor_scalar(
        ae[:], alsb[:], 8, 56, mybir.AluOpType.mult, mybir.AluOpType.add,
    )
    ae3 = ae[:].bitcast(f8).rearrange("p (n a) -> p n a", a=NUM_APOT)

    # --- unpack bit-planes on DVE: all h0 halves, then all h1 halves ---------
    planes = [[None] * 8 for _ in range(2)]
    for h in range(2):
        for s in range(8):
            t = sb.tile([WORDS, HC], i32, tag=f"pl{h}_{s}", name=f"pl{h}_{s}")
            if s < 7:
                nc.vector.tensor_scalar(
                    t[:], wsbH[h][:], 6 - s, 0x40404040,
                    mybir.AluOpType.logical_shift_left,
                    mybir.AluOpType.bitwise_and,
                )
            else:
                nc.vector.tensor_scalar(
                    t[:], wsbH[h][:], 1, 0x40404040,
                    mybir.AluOpType.logical_shift_right,
                    mybir.AluOpType.bitwise_and,
                )
            planes[h][s] = t[:].bitcast(f8)

    # --- 256 matmuls in 64 concurrent triples -> psum96[32b+g, (h, n')] ------
    # xall is s-major: lhsT block for (s, c) at columns (s*4+c)*32.
    psum96 = psum.tile([Q, NS], f32)
    for h in range(2):
        for s in range(8):
            for c in range(4):
                blk = (s * 4 + c) * G
                lhsT = xall_sb[:, blk:blk + G]              # [128, 32] bf16
                for b in range(WBIT):
                    base = 4 * (b * NH) + c
                    rhs = planes[h][s][:, base:base + 4 * (NH - 1) + 1:4]
                    nc.tensor.matmul(
                        psum96[32 * b:32 * b + 32, h * NH:(h + 1) * NH],
                        lhsT,
                        rhs,
                        start=(h == 0 and s == 0 and c == 0),
                        stop=(h == 1 and s == 7 and c == 3),
                    )

    # --- tail: scaleT = sum_a 2^alpha (DVE after unpacks); then
    #     prod = (psum96 - S_g) * scaleT  (one fused DVE op, bf16 out) --------
    scA = sb.tile([Q, NS], f32)
    scT = sb.tile([Q, NS], f32)
    nc.vector.tensor_tensor(scA[:], ae3[:, :, 0], ae3[:, :, 1],
                            mybir.AluOpType.add)
    nc.vector.tensor_tensor(scT[:], scA[:], ae3[:, :, 2],
                            mybir.AluOpType.add)
    nc.vector.scalar_tensor_tensor(
        prod[0:Q, :], psum96[:], csb[0:Q, 0:1], scT[:],
        mybir.AluOpType.subtract, mybir.AluOpType.mult,
    )

    # --- reduce over q (+bias row) via ones-matmuls; ACT copy; DMA out -------
    po = psum.tile([128, NBLK], f32, tag="po", name="po")
    for c in range(NBLK):
        nc.tensor.matmul(
            po[:, c:c + 1], prod[:, c * 128:(c + 1) * 128], onesb[:],
            start=(c == 0), stop=(c == NBLK - 1),
        )
    osb = sb.tile([128, NBLK], f32)
    nc.scalar.copy(osb[:], po[:])
    nc.scalar.dma_start(out[:, :], osb[:])


def _get_nc():
    if "nc" not in _CACHE:
        nc = bacc.Bacc(
            "TRN2",
            target_bir_lowering=False,
            debug=False,
            enable_asserts=False,
            num_devices=1,
        )
        with tile.TileContext(nc) as tc:
            _build_kernel_body(tc)
        nc.compile()
        _CACHE["nc"] = nc
    return _CACHE["nc"]


def _prep_inputs(x, binaryWeight, alpha, bias):
    """Host-side shard + layout prep (no arithmetic beyond tiny x-side sums)."""
    x = np.asarray(x, dtype=np.float32).reshape(K)
    binaryWeight = np.asarray(binaryWeight, dtype=np.int32)
    alpha = np.asarray(alpha, dtype=np.int32)
    bias = np.asarray(bias, dtype=np.float32).reshape(N)

    # Block-diagonal lhsT bank, s-major: xall[w, (s*4+c)*32+g] = x for j=8c+s
    xallj = np.zeros((WORDS, 32, G), dtype=np.float32)  # [w, j, g]
    k = np.arange(K)
    g = k // GROUP
    sub = (k % GROUP) // 32
    j = k % 32
    xallj[4 * g + sub, j, g] = x
    order = np.array([8 * c + s for s in range(8) for c in range(4)])
    xall = xallj[:, order, :].reshape(WORDS, G * G).astype(ml_dtypes.bfloat16)

    # S_g per group, tiled over b -> rows q=32b+g
    sg = x.reshape(G, GROUP).sum(axis=1).astype(np.float32)
    cstv = np.zeros((128, 5), dtype=np.float32)
    cstv[:Q, 0] = np.tile(sg, WBIT)

    in_maps = []
    for cc in range(NCORES):
        nsl = slice(cc * NS, (cc + 1) * NS)
        bw3 = binaryWeight[:, :, nsl]  # [128, 3, 512]
        bwc = np.ascontiguousarray(
            bw3.reshape(WORDS, WBIT, 2, NH).transpose(1, 2, 0, 3)
        ).reshape(6 * WORDS, NH)
        # alpha[n', b, g, a] -> [q=32b+g, (n', a)] int8 (values 1..7)
        al = alpha[nsl]  # [512, 3, 32, 3]
        al = np.transpose(al, (1, 2, 0, 3)).reshape(Q, ACOLS).astype(np.int8)
        cstc = cstv.copy()
        cstc[:, 1:5] = bias[nsl].reshape(NBLK, 128).T
        br = bias[nsl].astype(ml_dtypes.bfloat16).reshape(1, NS)
        in_maps.append(
            {"bwc": bwc, "xall": xall, "alc": al, "cst": cstc, "brow": br}
        )
    return in_maps


def _run(inputs, trace=False, **kw):
    nc = _get_nc()
    in_maps = _prep_inputs(**inputs)
    res = run_bass_kernel_spmd(
        nc, in_maps, core_ids=list(range(NCORES)), trace=trace, **kw
    )
    outs = []
    for cc in range(NCORES):
        o = res.results[cc]["out"]  # [128, NBLK]
        outs.append(np.ascontiguousarray(o.T).reshape(NS))  # n' = blk*128 + p
    full = np.concatenate(outs).reshape(1, N).astype(np.float32)
    return full, res


def kernel(**inputs):
    out, _ = _run(inputs, trace=False)
    return out


# revision 11
# speedup vs baseline: 1.1630x; 1.1630x over previous
"""Trainium2 Bass kernel for nn_LutLinear (BCQ/LUT-quantized linear layer).

Math (K=4096, N=4096, WBIT=3, GROUP=128, APOT=3):
  bits[k, b, n]  = bit (k%32) of binaryWeight[k//32, b, n]
  B              = 2*bits - 1                        (in {-1, +1})
  scale[n, b, g] = sum_a 2^alpha[n, b, g, a]
  out[n] = sum_{g,b} scale[n,b,g] * (sum_{k in group g} x[k] * B[k,b,n]) + bias[n]

Strategy (tensor-parallel over N, 8 cores, N'=512 each):
  * Bit-unpack on DVE: one int32 tensor_scalar (shift + AND 0x40404040) per
    bit-in-byte position s yields FOUR bit-planes at once -- the masked int32,
    bitcast to fp8e4m3, holds 2.0*bit in each byte.  The DVE chain is the
    critical path; nothing else heavy may touch SBUF while it runs.
  * PE: 96 accumulating matmuls psum96[32b+g, n'] = 2*sum_k x_k*bit as
    32 CONCURRENT TRIPLES (M=32 col-tiling over b, ~227ns per triple),
    riding just behind the unpack chain.  xall is s-major so its first DMA
    slice unblocks the first triples.
  * scale: alpha as int8; ACT computes 8*a+56 (Copy w/ scale+bias), whose
    int8 bit pattern IS fp8e4m3 2^a; DVE sums the 3 APoT terms post-unpack.
  * Tail (transpose-free): prod[0:96] = (psum96 - S_g) * scaleT fused on DVE
    (bf16 out); prod row 96 = bias row (DMA'd); reduce over 97 partitions
    via 4 ones-matmuls po[n'chunk] = prod_chunk^T @ ones; ACT copy; DMA out.
"""

import os
import sys

for _p in ("/opt/trn_rl_repo", "/opt/pypackages"):
    if os.path.isdir(_p) and _p not in sys.path:
        sys.path.insert(0, _p)

from contextlib import ExitStack

import ml_dtypes
import numpy as np

import concourse.bass as bass
import concourse.tile as tile
from concourse import bacc, mybir
from concourse._compat import with_exitstack
from concourse.bass_utils import run_bass_kernel_spmd

K = 4096
N = 4096
GROUP = 128
WBIT = 3
NUM_APOT = 3
G = K // GROUP          # 32 groups
NCORES = 8
NS = N // NCORES        # 512 output features per core
NBLK = NS // 128        # 4 partition-blocks of n'
WORDS = K // 32         # 128 packed words per (b, n)
Q = WBIT * G            # 96 (b, g) rows
WC = WBIT * NS          # 1536 packed-word columns
ACOLS = NS * NUM_APOT   # alpha cols per q-row: (a, n) a-major

_CACHE = {}


@with_exitstack
def _build_kernel_body(ctx: ExitStack, tc):
    nc = tc.nc
    f32 = mybir.dt.float32
    i32 = mybir.dt.int32
    i8 = mybir.dt.int8
    bf16 = mybir.dt.bfloat16
    f8 = mybir.dt.float8e4

    bw = nc.dram_tensor("bw", [WORDS, WC], i32, kind="ExternalInput")
    xall = nc.dram_tensor("xall", [WORDS, G * G], bf16, kind="ExternalInput")
    alc = nc.dram_tensor("alc", [Q, ACOLS], i8, kind="ExternalInput")
    cst = nc.dram_tensor("cst", [128, 5], f32, kind="ExternalInput")
    brow = nc.dram_tensor("brow", [1, NS], bf16, kind="ExternalInput")
    out = nc.dram_tensor("out", [128, NBLK], f32, kind="ExternalOutput")

    sb = ctx.enter_context(tc.tile_pool(name="sb", bufs=1))
    psum = ctx.enter_context(tc.tile_pool(name="psum", bufs=1, space="PSUM"))

    wsb = sb.tile([WORDS, WC], i32)
    xall_sb = sb.tile([WORDS, G * G], bf16)
    alsb = sb.tile([Q, ACOLS], i8)
    csb = sb.tile([128, 5], f32)
    prod = sb.tile([Q + 1, NS], bf16)   # rows 0..95 DVE; row 96 = bias (DMA)

    # --- DMAs: bw thirds first on each queue (they gate the unpack) ----------
    nc.sync.dma_start(wsb[:, 0:512], bw[:, 0:512])
    nc.scalar.dma_start(wsb[:, 512:1024], bw[:, 512:1024])
    nc.gpsimd.dma_start(wsb[:, 1024:1536], bw[:, 1024:1536])

    nc.sync.dma_start(xall_sb[:, 0:384], xall[:, 0:384])          # s=0..2
    nc.scalar.dma_start(xall_sb[:, 384:1024], xall[:, 384:1024])  # s=3..7
    nc.gpsimd.dma_start(alsb[:], alc[:, :])
    nc.sync.dma_start(csb[:], cst[:, :])
    nc.sync.dma_start(prod[Q:Q + 1, :], brow[:, :])

    onesb = sb.tile([Q + 1, 1], bf16)
    nc.vector.memset(onesb[:], 1.0)
    warm = sb.tile([128, 544], bf16)
    nc.vector.memset(warm[:], 0.0)

    # --- PE pre-warm to the stream start (HAM un-throttle + no re-throttle) --
    psw = psum.tile([32, NS], f32, tag="psw", name="psw")
    for _ in range(10):
        nc.tensor.matmul(
            psw[:, :], warm[:, :32], warm[:, 32:544], start=True, stop=True
        )

    # --- scale step 1 on ACT: ae = 8*alpha + 56 == fp8e4m3 pattern of 2^a ----
    ae = sb.tile([Q, ACOLS], i8)
    nc.scalar.activation(ae[:], alsb[:], mybir.ActivationFunctionType.Copy,
                         bias=56.0, scale=8.0)
    ae8 = ae[:].bitcast(f8)

    # --- unpack all 8 bit-planes on DVE --------------------------------------
    planes = []
    for s in range(8):
        t = sb.tile([WORDS, WC], i32, tag=f"pl{s}", name=f"pl{s}")
        if s < 7:
            nc.vector.tensor_scalar(
                t[:], wsb[:], 6 - s, 0x40404040,
                mybir.AluOpType.logical_shift_left,
                mybir.AluOpType.bitwise_and,
            )
        else:
            nc.vector.tensor_scalar(
                t[:], wsb[:], 1, 0x40404040,
                mybir.AluOpType.logical_shift_right,
                mybir.AluOpType.bitwise_and,
            )
        planes.append(t[:].bitcast(f8))

    # --- 96 matmuls in 32 concurrent triples -> psum96[32b+g, n'] ------------
    # xall is s-major: lhsT block for (s, c) at columns (s*4+c)*32.
    psum96 = psum.tile([Q, NS], f32)
    for s in range(8):
        for c in range(4):
            blk = (s * 4 + c) * G
            lhsT = xall_sb[:, blk:blk + G]                  # [128, 32] bf16
            for b in range(WBIT):
                base = 4 * (b * NS) + c
                rhs = planes[s][:, base:base + 4 * (NS - 1) + 1:4]
                nc.tensor.matmul(
                    psum96[32 * b:32 * b + 32, :],
                    lhsT,
                    rhs,
                    start=(s == 0 and c == 0),
                    stop=(s == 7 and c == 3),
                )

    # --- tail on DVE (after unpacks): scaleT = sum_a 2^alpha, then
    #     prod = (psum96 - S_g) * scaleT  (fused, bf16 out) --------------------
    scA = sb.tile([Q, NS], f32)
    scT = sb.tile([Q, NS], f32)
    nc.vector.tensor_tensor(scA[:], ae8[:, 0:NS], ae8[:, NS:2 * NS],
                            mybir.AluOpType.add)
    nc.vector.tensor_tensor(scT[:], scA[:], ae8[:, 2 * NS:3 * NS],
                            mybir.AluOpType.add)
    nc.vector.scalar_tensor_tensor(
        prod[0:Q, :], psum96[:], csb[0:Q, 0:1], scT[:],
        mybir.AluOpType.subtract, mybir.AluOpType.mult,
    )

    # --- reduce over q (+bias row) via ones-matmuls; ACT copy; DMA out -------
    po = psum.tile([128, NBLK], f32, tag="po", name="po")
    for c in range(NBLK):
        nc.tensor.matmul(
            po[:, c:c + 1], prod[:, c * 128:(c + 1) * 128], onesb[:],
            start=(c == 0), stop=(c == NBLK - 1),
        )
    osb = sb.tile([128, NBLK], f32)
    nc.scalar.copy(osb[:], po[:])
    nc.scalar.dma_start(out[:, :], osb[:])


def _get_nc():
    if "nc" not in _CACHE:
        nc = bacc.Bacc(
            "TRN2",
            target_bir_lowering=False,
            debug=False,
            enable_asserts=False,
            num_devices=1,
        )
        with tile.TileContext(nc) as tc:
            _build_kernel_body(tc)
        nc.compile()
        _CACHE["nc"] = nc
    return _CACHE["nc"]


def _prep_inputs(x, binaryWeight, alpha, bias):
    """Host-side shard + layout prep (no arithmetic beyond tiny x-side sums)."""
    x = np.asarray(x, dtype=np.float32).reshape(K)
    binaryWeight = np.asarray(binaryWeight, dtype=np.int32)
    alpha = np.asarray(alpha, dtype=np.int32)
    bias = np.asarray(bias, dtype=np.float32).reshape(N)

    # Block-diagonal lhsT bank, s-major: xall[w, (s*4+c)*32+g] = x for j=8c+s
    xallj = np.zeros((WORDS, 32, G), dtype=np.float32)  # [w, j, g]
    k = np.arange(K)
    g = k // GROUP
    sub = (k % GROUP) // 32
    j = k % 32
    xallj[4 * g + sub, j, g] = x
    order = np.array([8 * c + s for s in range(8) for c in range(4)])
    xall = xallj[:, order, :].reshape(WORDS, G * G).astype(ml_dtypes.bfloat16)

    # S_g per group, tiled over b -> rows q=32b+g
    sg = x.reshape(G, GROUP).sum(axis=1).astype(np.float32)
    cstv = np.zeros((128, 5), dtype=np.float32)
    cstv[:Q, 0] = np.tile(sg, WBIT)

    in_maps = []
    for cc in range(NCORES):
        nsl = slice(cc * NS, (cc + 1) * NS)
        bw_sh = np.ascontiguousarray(binaryWeight[:, :, nsl]).reshape(
            WORDS, WC
        )
        # alpha[n', b, g, a] -> [q=32b+g, (a, n')] int8 (a-major: contiguous
        # 512-col slices per APoT term for fast DVE adds)
        al = alpha[nsl]  # [512, 3, 32, 3]
        al = np.transpose(al, (1, 2, 3, 0)).reshape(Q, ACOLS).astype(np.int8)
        cstc = cstv.copy()
        cstc[:, 1:5] = bias[nsl].reshape(NBLK, 128).T
        br = bias[nsl].astype(ml_dtypes.bfloat16).reshape(1, NS)
        in_maps.append(
            {"bw": bw_sh, "xall": xall, "alc": al, "cst": cstc, "brow": br}
        )
    return in_maps


def _run(inputs, trace=False, **kw):
    nc = _get_nc()
    in_maps = _prep_inputs(**inputs)
    res = run_bass_kernel_spmd(
        nc, in_maps, core_ids=list(range(NCORES)), trace=trace, **kw
    )
    outs = []
    for cc in range(NCORES):
        o = res.results[cc]["out"]  # [128, NBLK]
        outs.append(np.ascontiguousarray(o.T).reshape(NS))  # n' = blk*128 + p
    full = np.concatenate(outs).reshape(1, N).astype(np.float32)
    return full, res


def kernel(**inputs):
    out, _ = _run(inputs, trace=False)
    return out
